# revision 19
# baseline (speedup 1.0000x reference)
"""Chamfer L1 distance kernel for Trainium2 (8 NeuronCores).

Full inputs: pred [4, 8192, 3] f32, target [4, 8192, 3] f32.
Output: scalar f32 = mean over batch of (sum_i min_j d(i,j) + sum_j min_i d(i,j)),
d = L1 distance.

Sharding: 8 cores = 4 batches x 2 pred-halves. Each core handles its 4096 preds
vs all 8192 targets and outputs:
  - rowmin [128, 32] bf16: rowmin[p, b] = min over all j of dist for pred
    (block b, partition p)
  - colmin [128, 8192] bf16: colmin[p, j] = min over this core's pred blocks
    (partition-p lane) of dist
Host finishes the reductions (min over partitions/core-pairs, sums, /B).

Device pipeline per 128-pred block b (j in 4096-wide act-chunks, DVE ops on
2048-wide slices):
  A_d = |T_d - p_d|   scalar.activation(Abs, bias=-p_d)  f32 -> bf16   (ACT x3)
  S01 = A0 + A1       vector.tensor_tensor add  bf16 2x               (DVE)
  S   = S01 + A2      vector.tensor_tensor add  bf16 2x               (DVE)
  rowacc = min(rowacc, S) across chunks; one tensor_reduce per block  (DVE)
  colmin chunk = min(colmin, S)                                       (DVE)
T_d are the target coords broadcast across all 128 partitions with a single
stride-0 broadcast DMA each. Intermediates are bf16 (end-to-end rel err vs the
fp32 reference ~3e-5); inputs stay f32 so no precision is lost in t - p.
"""

import sys

sys.path.insert(0, "/opt/trn_rl_repo")

import numpy as np

N_CORES = 8
B, N, M = 4, 8192, 8192
P = 128
NPRED = N // 2  # preds per core
NBLK = NPRED // P  # 32
CH = 2048  # j-chunk size
NCH = M // CH

_compiled = None


def _build(reps=1, wbufs=3, act_ch=4096):
    import concourse.bacc as bacc
    import concourse.mybir as mybir
    import concourse.tile as tile

    f32 = mybir.dt.float32
    bf16 = mybir.dt.bfloat16
    Alu = mybir.AluOpType
    Act = mybir.ActivationFunctionType

    nc = bacc.Bacc("TRN2", debug=False, num_devices=N_CORES)
    pred_rn = nc.dram_tensor("pred_rn", [P, NBLK * 3], f32, kind="ExternalInput").ap()
    target_t = nc.dram_tensor("target_t", [3, M], f32, kind="ExternalInput").ap()
    rowmin_d = nc.dram_tensor("rowmin", [P, NBLK], bf16, kind="ExternalOutput").ap()
    colmin_d = nc.dram_tensor("colmin", [P, M], bf16, kind="ExternalOutput").ap()

    BIG = 3.0e38

    with tile.TileContext(nc) as tc:
        abufs = 2 if act_ch > CH else wbufs
        with (
            tc.tile_pool(name="const", bufs=1) as cpool,
            tc.tile_pool(name="work", bufs=wbufs) as wpool,
            tc.tile_pool(name="apool", bufs=abufs) as apool,
        ):
            PNt = cpool.tile([P, NBLK * 3], f32, tag="PN")
            nc.sync.dma_start(PNt[:, :], pred_rn[:, :])

            T = [cpool.tile([P, M], f32, tag=f"T{d}", name=f"T{d}") for d in range(3)]
            for d in range(3):
                nc.sync.dma_start(
                    T[d][:, :], target_t[d : d + 1, :].broadcast_to([P, M])
                )

            colmin = cpool.tile([P, M], bf16, tag="colmin")
            nc.vector.memset(colmin[:, :], BIG)
            rowmin = cpool.tile([P, NBLK], bf16, tag="rowmin")
            rowacc = cpool.tile([P, CH], bf16, tag="rowacc")

            import contextlib

            loop_ctx = tc.For_i(0, reps, 1) if reps > 1 else contextlib.nullcontext()
            with loop_ctx:
              for b in range(NBLK):
                n0 = PNt[:, 3 * b : 3 * b + 1]
                n1 = PNt[:, 3 * b + 1 : 3 * b + 2]
                n2 = PNt[:, 3 * b + 2 : 3 * b + 3]
                for ac in range(M // act_ch):
                    ajs = slice(ac * act_ch, (ac + 1) * act_ch)
                    A0 = apool.tile([P, act_ch], bf16, tag="A0")
                    nc.scalar.activation(A0[:, :], T[0][:, ajs], Act.Abs, bias=n0, scale=1.0)
                    A1 = apool.tile([P, act_ch], bf16, tag="A1")
                    nc.scalar.activation(A1[:, :], T[1][:, ajs], Act.Abs, bias=n1, scale=1.0)
                    A2 = apool.tile([P, act_ch], bf16, tag="A2")
                    nc.scalar.activation(A2[:, :], T[2][:, ajs], Act.Abs, bias=n2, scale=1.0)
                    for c2 in range(act_ch // CH):
                        c = ac * (act_ch // CH) + c2
                        js = slice(c * CH, (c + 1) * CH)
                        cs = slice(c2 * CH, (c2 + 1) * CH)
                        S01 = wpool.tile([P, CH], bf16, tag="S01")
                        nc.vector.tensor_tensor(S01[:, :], A0[:, cs], A1[:, cs], Alu.add)
                        S = wpool.tile([P, CH], bf16, tag="S")
                        nc.vector.tensor_tensor(S[:, :], S01[:, :], A2[:, cs], Alu.add)
                        if c == 0:
                            nc.vector.tensor_copy(rowacc[:, :], S[:, :])
                        else:
                            nc.vector.tensor_tensor(
                                rowacc[:, :], rowacc[:, :], S[:, :], Alu.min
                            )
                        if c == NCH - 1:
                            nc.vector.tensor_reduce(
                                rowmin[:, b : b + 1],
                                rowacc[:, :],
                                mybir.AxisListType.X,
                                Alu.min,
                            )
                        nc.vector.tensor_tensor(
                            colmin[:, js], colmin[:, js], S[:, :], Alu.min
                        )

            nc.sync.dma_start(rowmin_d[:, :], rowmin[:, :])
            nc.sync.dma_start(colmin_d[:, :], colmin[:, :])

    nc.compile()
    return nc


def _shard(pred, target):
    in_maps = []
    for c in range(N_CORES):
        b, h = c // 2, c % 2
        pr = pred[b, h * NPRED : (h + 1) * NPRED, :]  # [4096, 3]
        prn = np.ascontiguousarray(
            -pr.reshape(NBLK, P, 3).transpose(1, 0, 2).reshape(P, NBLK * 3)
        )
        tt = np.ascontiguousarray(target[b].T)  # [3, 8192]
        in_maps.append({"pred_rn": prn, "target_t": tt})
    return in_maps


def _combine(results):
    total = 0.0
    for b in range(B):
        bwd = None
        for r in (results[2 * b], results[2 * b + 1]):
            rm = np.asarray(r["rowmin"]).astype(np.float32)  # [128, 32]
            total += float(rm.sum(dtype=np.float64))
            cm = np.asarray(r["colmin"]).astype(np.float32).min(axis=0)  # [8192]
            bwd = cm if bwd is None else np.minimum(bwd, cm)
        total += float(bwd.sum(dtype=np.float64))
    return np.float32(total / B)


def kernel(pred, target):
    global _compiled
    from concourse import bass_utils

    pred = np.asarray(pred, dtype=np.float32)
    target = np.asarray(target, dtype=np.float32)
    if _compiled is None:
        _compiled = _build()
    in_maps = _shard(pred, target)
    res = bass_utils.run_bass_kernel_spmd(
        _compiled, in_maps, core_ids=list(range(N_CORES))
    )
    return _combine(res.results)


# revision 20
# speedup vs baseline: 1.0007x; 1.0007x over previous
"""Chamfer L1 distance kernel for Trainium2 (8 NeuronCores).

Full inputs: pred [4, 8192, 3] f32, target [4, 8192, 3] f32.
Output: scalar f32 = mean over batch of (sum_i min_j d(i,j) + sum_j min_i d(i,j)),
d = L1 distance.

Sharding: 8 cores = 4 batches x 2 pred-halves. Each core handles its 4096 preds
vs all 8192 targets and outputs:
  - rowmin [128, 32] bf16: rowmin[p, b] = min over all j of dist for pred
    (block b, partition p)
  - colmin [128, 8192] bf16: colmin[p, j] = min over this core's pred blocks
    (partition-p lane) of dist
Host finishes the reductions (min over partitions/core-pairs, sums, /B).

Device pipeline per 128-pred block b (j in 4096-wide act-chunks, DVE ops on
2048-wide slices):
  A_d = |T_d - p_d|   scalar.activation(Abs, bias=-p_d)  f32 -> bf16   (ACT x3)
  S01 = A0 + A1       vector.tensor_tensor add  bf16 2x               (DVE)
  S   = S01 + A2      vector.tensor_tensor add  bf16 2x               (DVE)
  rowacc = min(rowacc, S) across chunks; one tensor_reduce per block  (DVE)
  colmin chunk = min(colmin, S)                                       (DVE)
T_d are the target coords broadcast across all 128 partitions with a single
stride-0 broadcast DMA each. Intermediates are bf16 (end-to-end rel err vs the
fp32 reference ~3e-5); inputs stay f32 so no precision is lost in t - p.
"""

import sys

sys.path.insert(0, "/opt/trn_rl_repo")

import numpy as np

N_CORES = 8
B, N, M = 4, 8192, 8192
P = 128
NPRED = N // 2  # preds per core
NBLK = NPRED // P  # 32
CH = 2048  # j-chunk size
NCH = M // CH

_compiled = None


def _build(reps=1, wbufs=4, act_ch=4096):
    import concourse.bacc as bacc
    import concourse.mybir as mybir
    import concourse.tile as tile

    f32 = mybir.dt.float32
    bf16 = mybir.dt.bfloat16
    Alu = mybir.AluOpType
    Act = mybir.ActivationFunctionType

    nc = bacc.Bacc("TRN2", debug=False, num_devices=N_CORES)
    pred_rn = nc.dram_tensor("pred_rn", [P, NBLK * 3], f32, kind="ExternalInput").ap()
    target_t = nc.dram_tensor("target_t", [3, M], f32, kind="ExternalInput").ap()
    rowmin_d = nc.dram_tensor("rowmin", [P, NBLK], bf16, kind="ExternalOutput").ap()
    colmin_d = nc.dram_tensor("colmin", [P, M], bf16, kind="ExternalOutput").ap()

    BIG = 3.0e38

    with tile.TileContext(nc) as tc:
        abufs = 2 if act_ch > CH else wbufs
        with (
            tc.tile_pool(name="const", bufs=1) as cpool,
            tc.tile_pool(name="work", bufs=wbufs) as wpool,
            tc.tile_pool(name="apool", bufs=abufs) as apool,
        ):
            PNt = cpool.tile([P, NBLK * 3], f32, tag="PN")
            nc.sync.dma_start(PNt[:, :], pred_rn[:, :])

            T = [cpool.tile([P, M], f32, tag=f"T{d}", name=f"T{d}") for d in range(3)]
            for d in range(3):
                nc.sync.dma_start(
                    T[d][:, :], target_t[d : d + 1, :].broadcast_to([P, M])
                )

            colmin = cpool.tile([P, M], bf16, tag="colmin")
            nc.vector.memset(colmin[:, :], BIG)
            rowmin = cpool.tile([P, NBLK], bf16, tag="rowmin")
            rowacc = cpool.tile([P, CH], bf16, tag="rowacc")

            import contextlib

            loop_ctx = tc.For_i(0, reps, 1) if reps > 1 else contextlib.nullcontext()
            with loop_ctx:
              for b in range(NBLK):
                n0 = PNt[:, 3 * b : 3 * b + 1]
                n1 = PNt[:, 3 * b + 1 : 3 * b + 2]
                n2 = PNt[:, 3 * b + 2 : 3 * b + 3]
                for ac in range(M // act_ch):
                    ajs = slice(ac * act_ch, (ac + 1) * act_ch)
                    A0 = apool.tile([P, act_ch], bf16, tag="A0")
                    nc.scalar.activation(A0[:, :], T[0][:, ajs], Act.Abs, bias=n0, scale=1.0)
                    A1 = apool.tile([P, act_ch], bf16, tag="A1")
                    nc.scalar.activation(A1[:, :], T[1][:, ajs], Act.Abs, bias=n1, scale=1.0)
                    A2 = apool.tile([P, act_ch], bf16, tag="A2")
                    nc.scalar.activation(A2[:, :], T[2][:, ajs], Act.Abs, bias=n2, scale=1.0)
                    for c2 in range(act_ch // CH):
                        c = ac * (act_ch // CH) + c2
                        js = slice(c * CH, (c + 1) * CH)
                        cs = slice(c2 * CH, (c2 + 1) * CH)
                        S01 = wpool.tile([P, CH], bf16, tag="S01")
                        nc.vector.tensor_tensor(S01[:, :], A0[:, cs], A1[:, cs], Alu.add)
                        S = wpool.tile([P, CH], bf16, tag="S")
                        nc.vector.tensor_tensor(S[:, :], S01[:, :], A2[:, cs], Alu.add)
                        if c == 0:
                            nc.vector.tensor_copy(rowacc[:, :], S[:, :])
                        else:
                            nc.vector.tensor_tensor(
                                rowacc[:, :], rowacc[:, :], S[:, :], Alu.min
                            )
                        if c == NCH - 1:
                            nc.vector.tensor_reduce(
                                rowmin[:, b : b + 1],
                                rowacc[:, :],
                                mybir.AxisListType.X,
                                Alu.min,
                            )
                        nc.vector.tensor_tensor(
                            colmin[:, js], colmin[:, js], S[:, :], Alu.min
                        )

            nc.sync.dma_start(rowmin_d[:, :], rowmin[:, :])
            nc.sync.dma_start(colmin_d[:, :], colmin[:, :])

    nc.compile()
    return nc


def _shard(pred, target):
    in_maps = []
    for c in range(N_CORES):
        b, h = c // 2, c % 2
        pr = pred[b, h * NPRED : (h + 1) * NPRED, :]  # [4096, 3]
        prn = np.ascontiguousarray(
            -pr.reshape(NBLK, P, 3).transpose(1, 0, 2).reshape(P, NBLK * 3)
        )
        tt = np.ascontiguousarray(target[b].T)  # [3, 8192]
        in_maps.append({"pred_rn": prn, "target_t": tt})
    return in_maps


def _combine(results):
    total = 0.0
    for b in range(B):
        bwd = None
        for r in (results[2 * b], results[2 * b + 1]):
            rm = np.asarray(r["rowmin"]).astype(np.float32)  # [128, 32]
            total += float(rm.sum(dtype=np.float64))
            cm = np.asarray(r["colmin"]).astype(np.float32).min(axis=0)  # [8192]
            bwd = cm if bwd is None else np.minimum(bwd, cm)
        total += float(bwd.sum(dtype=np.float64))
    return np.float32(total / B)


def kernel(pred, target):
    global _compiled
    from concourse import bass_utils

    pred = np.asarray(pred, dtype=np.float32)
    target = np.asarray(target, dtype=np.float32)
    if _compiled is None:
        _compiled = _build()
    in_maps = _shard(pred, target)
    res = bass_utils.run_bass_kernel_spmd(
        _compiled, in_maps, core_ids=list(range(N_CORES))
    )
    return _combine(res.results)
